# revision 26
# baseline (speedup 1.0000x reference)
"""MoE (top-2, capacity-dropped) Trainium2 kernel, expert-parallel across 8 NeuronCores.

Strategy:
  - Each core receives: full `features`, gate weights with the expert axis
    PERMUTED so that its own 2 experts are columns 0..1, and only its own
    2 experts' W1/b1/W2/b2 slices.
  - On device: fp32 router (scores, top-2, softmax weights, capacity
    positions via matmul-cumsum), dispatch via dma_gather (transposed,
    bf16), expert FFN in bf16 (fp32 accum), weighted combine via
    dma_scatter_add into a [B,O] fp32 partial, ReduceScatter across the
    8 cores, each core emits its row-shard of the summed output.
  - Host concatenates the 8 shards.
"""

import sys

for _p in ("/opt/trn_rl_repo", "/opt/pypackages"):
    if _p not in sys.path:
        sys.path.append(_p)

import numpy as np

from concourse import bass, mybir, tile, library_config
from concourse import bacc

FP32 = mybir.dt.float32
BF16 = mybir.dt.bfloat16
I32 = mybir.dt.int32
I16 = mybir.dt.int16


def build_moe(B=16384, F=1024, H=4096, O=1024, E=16, NCORES=8, CAP=2560,
              SBLK=512, CHUNK=512, partial_dtype=FP32, debug_outputs=False):
    EL = E // NCORES              # experts per core
    NT = B // 128                 # token tiles
    NCH = B // CHUNK              # router chunks
    TPC = CHUNK // 128            # token tiles per chunk
    FC = F // 128
    HC = H // 128
    OC = O // 128
    NBLK = CAP // SBLK            # slot blocks per expert
    G = SBLK // 128
    S = EL * CAP                  # total slots per core
    SW16 = S // 16
    SW128 = S // 128
    BP = B + 128                  # partial rows (dump row at B)
    RSR = BP // NCORES            # ReduceScatter rows per core
    assert BP % NCORES == 0 and NT % 8 == 0 and CAP % SBLK == 0 and SBLK % 128 == 0
    HQ = max(1, H // 512)         # 512-wide column chunks for W1 conversion
    W1CW = min(H, 512)
    WCONVW = max(O, W1CW)

    nc = bacc.Bacc("TRN2", target_bir_lowering=False, debug=False,
                   num_devices=NCORES)

    # ---- I/O -------------------------------------------------------------
    features = nc.dram_tensor("features", [B, F], FP32, kind="ExternalInput")
    Wg = nc.dram_tensor("Wg", [E, F], FP32, kind="ExternalInput")
    bg = nc.dram_tensor("bg", [1, E], FP32, kind="ExternalInput")
    eb = nc.dram_tensor("eb", [1, E], FP32, kind="ExternalInput")
    W1 = nc.dram_tensor("W1", [EL, F, H], FP32, kind="ExternalInput")
    b1 = nc.dram_tensor("b1", [EL, H], FP32, kind="ExternalInput")
    W2 = nc.dram_tensor("W2", [EL, H, O], FP32, kind="ExternalInput")
    b2 = nc.dram_tensor("b2", [EL, O], FP32, kind="ExternalInput")
    out = nc.dram_tensor("out", [RSR, O], FP32, kind="ExternalOutput")
    if debug_outputs:
        dbg_inv = nc.dram_tensor("dbg_inv", [16, SW16], I16, kind="ExternalOutput")
        dbg_w = nc.dram_tensor("dbg_w", [128, EL], FP32, kind="ExternalOutput")
        dbg_part = nc.dram_tensor("dbg_part", [128, O], partial_dtype, kind="ExternalOutput")
        dbg_fbf = nc.dram_tensor("dbg_fbf", [128, F], BF16, kind="ExternalOutput")
        dbg_cums = nc.dram_tensor("dbg_cums", [128, 8, 2], FP32, kind="ExternalOutput")
        dbg_wsb = nc.dram_tensor("dbg_wsb", [128, 8, 2], FP32, kind="ExternalOutput")

    # ---- internal DRAM ---------------------------------------------------
    features_bf = nc.dram_tensor("features_bf", [B + 128, F], BF16, kind="Internal")
    w2bf = nc.dram_tensor("w2bf", [EL, OC, 128, HC, 128], BF16, kind="Internal")
    partial = nc.dram_tensor("partial", [BP, O], partial_dtype, kind="Internal")
    inv_lin = nc.dram_tensor("inv_lin", [S, 1], I16, kind="Internal")
    w_dram = nc.dram_tensor("w_dram", [B + 128, EL], FP32, kind="Internal")
    rs_out = nc.dram_tensor("rs_out", [RSR, O], partial_dtype, kind="Internal")

    with tile.TileContext(nc) as tc:
        lp = tc.alloc_tile_pool(name="longlived", bufs=1)
        pp = tc.alloc_tile_pool(name="psum_small", bufs=1, space="PSUM")
        mid = tc.alloc_tile_pool(name="midlived", bufs=1)

        # ---- phase 0: constants / prologue -------------------------------
        with tc.tile_critical():
            ids_all = mid.tile([128, NT, EL], I16)
            nc.gpsimd.iota(ids_all[:], base=0, channel_multiplier=1,
                           pattern=[[128, NT], [0, EL]])
            nc.gpsimd.load_library(library_config.mlp)

        ident = mid.tile([128, 128], FP32)
        nc.vector.memset(ident[:], 0.0)
        nc.gpsimd.affine_select(out=ident[:], in_=ident[:],
                                compare_op=mybir.AluOpType.not_equal, fill=1.0,
                                base=0, channel_multiplier=1, pattern=[[-1, 128]])
        ident_bf = lp.tile([128, 128], BF16)
        nc.vector.tensor_copy(out=ident_bf[:], in_=ident[:])
        # L[x,y] = 1 if x <= y (inclusive-cumsum matmul weight)
        L = mid.tile([128, 128], FP32)
        nc.vector.memset(L[:], 0.0)
        nc.gpsimd.affine_select(out=L[:], in_=L[:],
                                compare_op=mybir.AluOpType.is_gt, fill=1.0,
                                base=0, channel_multiplier=1, pattern=[[-1, 128]])
        ones_col = mid.tile([128, 1], FP32)
        nc.vector.memset(ones_col[:], 1.0)
        ones_row = mid.tile([1, 128], FP32)
        nc.vector.memset(ones_row[:], 1.0)
        ones_sblk_bf = lp.tile([1, SBLK], BF16)
        nc.vector.memset(ones_sblk_bf[:], 1.0)

        # gate bias row = bg + expert_bias
        gb = mid.tile([1, E], FP32)
        bg_sb = mid.tile([1, E], FP32)
        eb_sb = mid.tile([1, E], FP32)
        nc.sync.dma_start(out=bg_sb[:], in_=bg[:, :])
        nc.sync.dma_start(out=eb_sb[:], in_=eb[:, :])
        nc.vector.tensor_tensor(out=gb[:], in0=bg_sb[:], in1=eb_sb[:],
                                op=mybir.AluOpType.add)

        # WgT [128, FC, E]
        wg_sb = mid.tile([E, F], FP32)
        nc.sync.dma_start(out=wg_sb[:], in_=Wg[:, :])
        WgT = mid.tile([128, FC, E], FP32)
        for fc in range(FC):
            tps = pp.tile([128, E], FP32, tag="wgt_ps")
            nc.tensor.transpose(out=tps[:], in_=wg_sb[:, fc * 128:(fc + 1) * 128],
                                identity=ident[0:E, 0:E])
            nc.vector.tensor_copy(out=WgT[:, fc, :], in_=tps[:])

        # zero-init partial, dump rows of features_bf, prefills
        with tc.tile_pool(name="prolog", bufs=1) as prol:
            zt = prol.tile([128, O], partial_dtype)
            nc.vector.memset(zt[:], 0.0)
            for r in range(BP // 128):
                nc.sync.dma_start(out=partial[r * 128:(r + 1) * 128, :], in_=zt[:])
            zbf = prol.tile([128, F], BF16)
            nc.vector.memset(zbf[:], 0.0)
            nc.scalar.dma_start(out=features_bf[B:B + 128, :], in_=zbf[:])
            pf = prol.tile([128, SW128], I16)
            nc.vector.memset(pf[:], B)
            nc.gpsimd.dma_start(
                out=inv_lin.ap().rearrange("(a b) c -> a (b c)", a=128), in_=pf[:])
            zw = prol.tile([128, EL], FP32)
            nc.vector.memset(zw[:], 0.0)
            nc.gpsimd.dma_start(out=w_dram[B:B + 128, :], in_=zw[:])

        # router state kept across phases
        w_sb = mid.tile([128, NT, EL], FP32)
        cums_sb = mid.tile([128, NT, EL], FP32)
        assign_sb = mid.tile([128, NT, EL], FP32)
        tpp = tc.alloc_tile_pool(name="tot_ps", bufs=1, space="PSUM")
        tot_ps = tpp.tile([EL, NT], FP32)

        # ---- phase 1: router --------------------------------------------
        with tc.tile_pool(name="router_sb", bufs=2) as rsb, \
             tc.tile_pool(name="router_ps", bufs=2, space="PSUM") as rps:
            for c in range(NCH):
                t0 = c * CHUNK
                XT = rsb.tile([128, FC, CHUNK], FP32, tag="XT")
                for ti in range(TPC):
                    ft = rsb.tile([128, F], FP32, tag="ft")
                    r0 = t0 + ti * 128
                    nc.sync.dma_start(out=ft[:], in_=features[r0:r0 + 128, :])
                    fbf = rsb.tile([128, F], BF16, tag="fbf")
                    nc.vector.tensor_copy(out=fbf[:], in_=ft[:])
                    nc.scalar.dma_start(out=features_bf[r0:r0 + 128, :], in_=fbf[:])
                    for fc in range(FC):
                        xps = rps.tile([128, 128], FP32, tag="xps")
                        nc.tensor.transpose(out=xps[:],
                                            in_=ft[:, fc * 128:(fc + 1) * 128],
                                            identity=ident[:])
                        nc.vector.tensor_copy(
                            out=XT[:, fc, ti * 128:(ti + 1) * 128], in_=xps[:])
                for ti in range(TPC):
                    T = c * TPC + ti
                    scp = rps.tile([128, E], FP32, tag="scp")
                    for fc in range(FC):
                        nc.tensor.matmul(out=scp[:],
                                         lhsT=XT[:, fc, ti * 128:(ti + 1) * 128],
                                         rhs=WgT[:, fc, :],
                                         start=(fc == 0), stop=False)
                    nc.tensor.matmul(out=scp[:], lhsT=ones_row[:], rhs=gb[:],
                                     start=False, stop=True)
                    sc = rsb.tile([128, E], FP32, tag="sc")
                    nc.vector.tensor_copy(out=sc[:], in_=scp[:])
                    m8 = rsb.tile([128, 8], FP32, tag="m8")
                    nc.vector.max(out=m8[:], in_=sc[:])
                    nm1 = rsb.tile([128, 1], FP32, tag="nm1")
                    nc.vector.tensor_scalar_mul(nm1[:], m8[:, 0:1], -1.0)
                    # d = 1 + exp(m2 - m1); rd = 1/d
                    e2 = rsb.tile([128, 1], FP32, tag="e2")
                    nc.scalar.activation(out=e2[:], in_=m8[:, 1:2],
                                         func=mybir.ActivationFunctionType.Exp,
                                         bias=nm1[:, 0:1], scale=1.0)
                    d = rsb.tile([128, 1], FP32, tag="d")
                    nc.vector.tensor_scalar_add(d[:], e2[:], 1.0)
                    rd = rsb.tile([128, 1], FP32, tag="rd")
                    nc.vector.reciprocal(out=rd[:], in_=d[:])
                    # local-expert weights and assignment
                    el_ = rsb.tile([128, EL], FP32, tag="el_")
                    nc.scalar.activation(out=el_[:], in_=sc[:, 0:EL],
                                         func=mybir.ActivationFunctionType.Exp,
                                         bias=nm1[:, 0:1], scale=1.0)
                    wl = rsb.tile([128, EL], FP32, tag="wl")
                    nc.vector.tensor_scalar_mul(wl[:], el_[:], rd[:, 0:1])
                    al = rsb.tile([128, EL], FP32, tag="al")
                    nc.vector.tensor_scalar(out=al[:], in0=sc[:, 0:EL],
                                            scalar1=m8[:, 1:2], scalar2=None,
                                            op0=mybir.AluOpType.is_ge)
                    nc.vector.tensor_tensor(out=w_sb[:, T, :], in0=wl[:],
                                            in1=al[:], op=mybir.AluOpType.mult)
                    nc.vector.tensor_copy(out=assign_sb[:, T, :], in_=al[:])
                    cmp_ = rps.tile([128, EL], FP32, tag="scp")
                    nc.tensor.matmul(out=cmp_[:], lhsT=L[:], rhs=al[:],
                                     start=True, stop=True)
                    nc.vector.tensor_copy(out=cums_sb[:, T, :], in_=cmp_[:])
                    nc.tensor.matmul(out=tot_ps[:, T:T + 1], lhsT=al[:],
                                     rhs=ones_col[:], start=True, stop=True)

        # ---- phase 2: capacity offsets ----------------------------------
        tot_sb = mid.tile([EL, NT], FP32)
        nc.vector.tensor_copy(out=tot_sb[:], in_=tot_ps[:])
        znt = mid.tile([EL, NT], FP32)
        nc.vector.memset(znt[:], 0.0)
        incl = mid.tile([EL, NT], FP32)
        nc.vector.tensor_tensor_scan(out=incl[:], data0=tot_sb[:], data1=znt[:],
                                     initial=0.0, op0=mybir.AluOpType.add,
                                     op1=mybir.AluOpType.add)
        excl = mid.tile([EL, NT], FP32)
        nc.vector.tensor_tensor(out=excl[:], in0=incl[:], in1=tot_sb[:],
                                op=mybir.AluOpType.subtract)
        tpp.release()
        off_rows = []
        for e in range(EL):
            orow = mid.tile([1, NT], FP32, tag=f"orow{e}")
            nc.gpsimd.dma_start(out=orow[:], in_=excl[e:e + 1, :])
            off_rows.append(orow)

        # ---- phase 3: slot scatter --------------------------------------
        # dense w table write: w_dram[T*128+p, e] = w_sb[p, T, e]
        nc.scalar.dma_start(
            out=w_dram.ap()[0:B, :].rearrange("(t p) e -> p t e", p=128),
            in_=w_sb[:, :, :])
        cbase = mid.tile([128, 8, EL], FP32)
        for e in range(EL):
            nc.vector.memset(cbase[:, :, e], float(e * CAP - 1))
        with tc.tile_pool(name="p3sb", bufs=2) as p3, \
             tc.tile_pool(name="p3ps", bufs=2, space="PSUM") as p3p:
            for b8 in range(NT // 8):
                Tb = b8 * 8
                offb = p3.tile([128, 8, EL], FP32, tag="offb")
                for e in range(EL):
                    bcp = p3p.tile([128, 8], FP32, tag="bcp")
                    nc.tensor.matmul(out=bcp[:], lhsT=ones_row[:],
                                     rhs=off_rows[e][0:1, Tb:Tb + 8],
                                     start=True, stop=True)
                    nc.vector.tensor_copy(out=offb[:, :, e], in_=bcp[:])
                gi = p3.tile([128, 8, EL], FP32, tag="gi")
                nc.vector.tensor_tensor(out=gi[:], in0=cums_sb[:, Tb:Tb + 8, :],
                                        in1=offb[:], op=mybir.AluOpType.add)
                le = p3.tile([128, 8, EL], FP32, tag="le")
                nc.vector.tensor_scalar(out=le[:], in0=gi[:], scalar1=float(CAP),
                                        scalar2=None, op0=mybir.AluOpType.is_le)
                kept = p3.tile([128, 8, EL], FP32, tag="kept")
                nc.vector.tensor_tensor(out=kept[:], in0=le[:],
                                        in1=assign_sb[:, Tb:Tb + 8, :],
                                        op=mybir.AluOpType.mult)
                slotg = p3.tile([128, 8, EL], FP32, tag="slotg")
                nc.vector.tensor_tensor(out=slotg[:], in0=gi[:], in1=cbase[:],
                                        op=mybir.AluOpType.add)
                kept8 = p3.tile([128, 8, EL], mybir.dt.uint8, tag="kept8")
                nc.vector.tensor_copy(out=kept8[:], in_=kept[:])
                slotm = p3.tile([128, 8, EL], FP32, tag="slotm")
                nc.vector.memset(slotm[:], 65535.0)
                nc.vector.copy_predicated(out=slotm[:], mask=kept8[:], data=slotg[:])
                sloti = p3.tile([128, 8, EL], I32, tag="sloti")
                nc.vector.tensor_copy(out=sloti[:], in_=slotm[:])
                for t8 in range(8):
                    for e in range(EL):
                        nc.gpsimd.indirect_dma_start(
                            out=inv_lin[:, :],
                            out_offset=bass.IndirectOffsetOnAxis(
                                ap=sloti[:, t8, e:e + 1], axis=0),
                            in_=ids_all[:, Tb + t8, e:e + 1], in_offset=None,
                            bounds_check=S - 1, oob_is_err=False)

        if debug_outputs:
            nc.gpsimd.dma_start(out=dbg_cums[:, :, :], in_=cums_sb[:, 0:8, :])
            nc.gpsimd.dma_start(out=dbg_wsb[:, :, :], in_=w_sb[:, 0:8, :])
        mid.release()

        if debug_outputs:
            with nc.allow_non_contiguous_dma(reason="tiny debug idx load"):
                nc.gpsimd.dma_start(
                    out=dbg_inv[:, :],
                    in_=inv_lin.ap().rearrange("(j p) c -> p (j c)", p=16))
            nc.gpsimd.dma_start(out=dbg_w[:, :], in_=w_dram[0:128, :])

        # ---- phase 4: wrapped index tile --------------------------------
        idx_all = lp.tile([128, SW16], I16)
        with nc.allow_non_contiguous_dma(reason="80KB wrapped idx load"):
            nc.gpsimd.dma_start(
                out=idx_all[0:16, :],
                in_=inv_lin.ap().rearrange("(j p) c -> p (j c)", p=16))
        nc.gpsimd.dma_start(out=idx_all[16:32, :], in_=idx_all[0:16, :])
        nc.gpsimd.dma_start(out=idx_all[32:64, :], in_=idx_all[0:32, :])
        nc.gpsimd.dma_start(out=idx_all[64:128, :], in_=idx_all[0:64, :])

        # ---- phase 5: experts -------------------------------------------
        with tc.tile_pool(name="exp_sb", bufs=1) as esb, \
             tc.tile_pool(name="exp_db", bufs=2) as edb, \
             tc.tile_pool(name="exp_ps", bufs=2, space="PSUM") as eps:
            for e in range(EL):
                # W1 -> SBUF (bf16), W2 -> DRAM (bf16, swizzled tiles)
                w1sb = esb.tile([128, FC, H], BF16, tag="w1sb")
                for fc in range(FC):
                    for hq in range(HQ):
                        wt = edb.tile([128, WCONVW], FP32, tag="wconv")
                        nc.sync.dma_start(
                            out=wt[:, 0:W1CW],
                            in_=W1[e, fc * 128:(fc + 1) * 128,
                                   hq * W1CW:(hq + 1) * W1CW])
                        nc.vector.tensor_copy(
                            out=w1sb[:, fc, hq * W1CW:(hq + 1) * W1CW],
                            in_=wt[:, 0:W1CW])
                for hc in range(HC):
                    wt = edb.tile([128, WCONVW], FP32, tag="wconv")
                    nc.sync.dma_start(out=wt[:, 0:O],
                                      in_=W2[e, hc * 128:(hc + 1) * 128, :])
                    wb = edb.tile([128, O], BF16, tag="w2convb")
                    nc.vector.tensor_copy(out=wb[:], in_=wt[:, 0:O])
                    nc.scalar.dma_start(
                        out=w2bf[e, :, :, hc, :].rearrange("oc p o -> p oc o"),
                        in_=wb[:].rearrange("p (oc o) -> p oc o", oc=OC))
                b1row = esb.tile([1, H], BF16, tag="b1row")
                b1f = esb.tile([1, H], FP32, tag="b1f")
                nc.sync.dma_start(out=b1f[:], in_=b1[e:e + 1, :])
                nc.vector.tensor_copy(out=b1row[:], in_=b1f[:])
                b2row = esb.tile([1, O], BF16, tag="b2row")
                b2f = esb.tile([1, O], FP32, tag="b2f")
                nc.sync.dma_start(out=b2f[:], in_=b2[e:e + 1, :])
                nc.vector.tensor_copy(out=b2row[:], in_=b2f[:])

                for blk in range(NBLK):
                    s0 = e * CAP + blk * SBLK
                    idxs = idx_all[:, s0 // 16:(s0 + SBLK) // 16]
                    bufT = edb.tile([128, FC, SBLK], BF16, tag="bufT")
                    nc.gpsimd.dma_gather(out_ap=bufT[:], in_ap=features_bf[:, :],
                                         idxs_ap=idxs, num_idxs=SBLK,
                                         num_idxs_reg=SBLK, elem_size=F,
                                         transpose=True)
                    hT = esb.tile([128, HC, SBLK], BF16, tag="hT")
                    for hc in range(HC):
                        ps = eps.tile([128, SBLK], FP32, tag="mmps")
                        for fc in range(FC):
                            nc.tensor.matmul(out=ps[:],
                                             lhsT=w1sb[:, fc, hc * 128:(hc + 1) * 128],
                                             rhs=bufT[:, fc, :],
                                             start=(fc == 0), stop=False)
                        nc.tensor.matmul(out=ps[:],
                                         lhsT=b1row[0:1, hc * 128:(hc + 1) * 128],
                                         rhs=ones_sblk_bf[:],
                                         start=False, stop=True)
                        nc.scalar.activation(out=hT[:, hc, :], in_=ps[:],
                                             func=mybir.ActivationFunctionType.Relu)
                    yT = esb.tile([128, OC, SBLK], BF16, tag="yT")
                    for oc in range(OC):
                        w2t = edb.tile([128, HC, 128], BF16, tag="w2t")
                        nc.sync.dma_start(
                            out=w2t[:],
                            in_=w2bf[e, oc, :, :, :].rearrange("p hc o -> p (hc o)")
                                .rearrange("p (hc o) -> p hc o", hc=HC))
                        ps2 = eps.tile([128, SBLK], FP32, tag="mmps")
                        for hc in range(HC):
                            nc.tensor.matmul(out=ps2[:], lhsT=w2t[:, hc, :],
                                             rhs=hT[:, hc, :],
                                             start=(hc == 0), stop=False)
                        nc.tensor.matmul(out=ps2[:],
                                         lhsT=b2row[0:1, oc * 128:(oc + 1) * 128],
                                         rhs=ones_sblk_bf[:],
                                         start=False, stop=True)
                        nc.scalar.activation(out=yT[:, oc, :], in_=ps2[:],
                                             func=mybir.ActivationFunctionType.Copy)
                    invt16 = edb.tile([128, G], I16, tag="invt16")
                    with nc.allow_non_contiguous_dma(reason="1KB slot idx load"):
                        nc.gpsimd.dma_start(
                            out=invt16[:],
                            in_=inv_lin.ap()[s0:s0 + SBLK, :]
                                .rearrange("(g p) c -> p (g c)", p=128))
                    invt = edb.tile([128, G], I32, tag="invt")
                    nc.vector.tensor_copy(out=invt[:], in_=invt16[:])
                    wpair = edb.tile([128, G, EL], FP32, tag="wpair")
                    for g_i in range(G):
                        nc.gpsimd.indirect_dma_start(
                            out=wpair[:, g_i, :], out_offset=None,
                            in_=w_dram[:, :],
                            in_offset=bass.IndirectOffsetOnAxis(
                                ap=invt[:, g_i:g_i + 1], axis=0))
                    wsl = edb.tile([128, G], FP32, tag="wsl")
                    nc.vector.tensor_copy(out=wsl[:], in_=wpair[:, :, e])
                    ysc = esb.tile([128, G, O], partial_dtype, tag="ysc")
                    for g_i in range(G):
                        for oc in range(OC):
                            tp = eps.tile([128, 128], BF16, tag="tpps")
                            nc.tensor.transpose(
                                out=tp[:],
                                in_=yT[:, oc, g_i * 128:(g_i + 1) * 128],
                                identity=ident_bf[:])
                            nc.vector.tensor_scalar_mul(
                                ysc[:, g_i, oc * 128:(oc + 1) * 128], tp[:],
                                wsl[:, g_i:g_i + 1])
                    nc.gpsimd.dma_scatter_add(out_ap=partial[:, :], in_ap=ysc[:],
                                              idxs_ap=idxs, num_idxs=SBLK,
                                              num_idxs_reg=SBLK, elem_size=O)

        if debug_outputs:
            nc.gpsimd.dma_start(out=dbg_part[:, :], in_=partial[0:128, :])
            nc.gpsimd.dma_start(out=dbg_fbf[:, :], in_=features_bf[0:128, :])

        # ---- phase 6: ReduceScatter + output ----------------------------
        nc.gpsimd.collective_compute(
            "ReduceScatter", mybir.AluOpType.add,
            replica_groups=[list(range(NCORES))],
            ins=[partial.ap().opt()], outs=[rs_out.ap().opt()])
        with tc.tile_pool(name="outp", bufs=2) as op_:
            for r in range(RSR // 128):
                ot = op_.tile([128, O], partial_dtype, tag="ot")
                nc.gpsimd.dma_start(out=ot[:], in_=rs_out[r * 128:(r + 1) * 128, :])
                if partial_dtype == FP32:
                    nc.sync.dma_start(out=out[r * 128:(r + 1) * 128, :], in_=ot[:])
                else:
                    of = op_.tile([128, O], FP32, tag="of")
                    nc.vector.tensor_copy(out=of[:], in_=ot[:])
                    nc.sync.dma_start(out=out[r * 128:(r + 1) * 128, :], in_=of[:])
            if RSR % 128:
                r0 = (RSR // 128) * 128
                rem = RSR - r0
                ot = op_.tile([128, O], partial_dtype, tag="ot")
                nc.gpsimd.dma_start(out=ot[0:rem, :], in_=rs_out[r0:RSR, :])
                if partial_dtype == FP32:
                    nc.sync.dma_start(out=out[r0:RSR, :], in_=ot[0:rem, :])
                else:
                    of = op_.tile([128, O], FP32, tag="of")
                    nc.vector.tensor_copy(out=of[0:rem, :], in_=ot[0:rem, :])
                    nc.sync.dma_start(out=out[r0:RSR, :], in_=of[0:rem, :])

        pp.release()
        lp.release()

    nc.compile()
    return nc


def make_in_maps(inputs, E=16, NCORES=8):
    """Shard the full inputs: permute gate expert axis per core, slice expert
    weights. Returns list of per-core input dicts."""
    EL = E // NCORES
    features = np.ascontiguousarray(inputs["features"], dtype=np.float32)
    Wg = np.asarray(inputs["Wg"], dtype=np.float32)
    bg = np.asarray(inputs["bg"], dtype=np.float32)
    eb = np.asarray(inputs["expert_bias"], dtype=np.float32)
    W1 = np.asarray(inputs["W1"], dtype=np.float32)
    b1 = np.asarray(inputs["b1"], dtype=np.float32)
    W2 = np.asarray(inputs["W2"], dtype=np.float32)
    b2 = np.asarray(inputs["b2"], dtype=np.float32)
    in_maps = []
    for i in range(NCORES):
        mine = list(range(i * EL, (i + 1) * EL))
        rest = [e for e in range(E) if e not in mine]
        perm = mine + rest
        in_maps.append({
            "features": features,
            "Wg": np.ascontiguousarray(Wg[perm]),
            "bg": np.ascontiguousarray(bg[perm].reshape(1, E)),
            "eb": np.ascontiguousarray(eb[perm].reshape(1, E)),
            "W1": np.ascontiguousarray(W1[mine]),
            "b1": np.ascontiguousarray(b1[mine]),
            "W2": np.ascontiguousarray(W2[mine]),
            "b2": np.ascontiguousarray(b2[mine]),
        })
    return in_maps


_NC_CACHE = {}


def kernel(**inputs):
    import os
    from concourse.bass_utils import run_bass_kernel_spmd
    B, F = 16384, 1024
    H, O, E, NCORES, CAP = 4096, 1024, 16, 8, 2560
    key = "full"
    if key not in _NC_CACHE:
        _NC_CACHE[key] = build_moe(B=B, F=F, H=H, O=O, E=E, NCORES=NCORES,
                                   CAP=CAP)
    nc = _NC_CACHE[key]
    in_maps = make_in_maps(inputs, E=E, NCORES=NCORES)
    res = run_bass_kernel_spmd(nc, in_maps, core_ids=list(range(NCORES)))
    shards = [res.results[i]["out"] for i in range(NCORES)]
    full = np.concatenate(shards, axis=0)[:B]
    return full.astype(np.float32)


if __name__ == "__main__":
    data = np.load("/root/problem/work/ref_data.npz")
    inputs = {k: data[k] for k in
              ["features", "Wg", "bg", "W1", "b1", "W2", "b2", "expert_bias"]}
    outp = kernel(**inputs)
    exp = data["expected"]
    err = np.linalg.norm(outp - exp) / np.linalg.norm(exp)
    print("Relative error:", err)
